# revision 19
# baseline (speedup 1.0000x reference)
"""Bucketed causal-linear attention kernel for Trainium2 (8 NeuronCores).

Sharding: core c handles batch b = c//2 and head-group hg = c%2 (8 of 16
heads). Each core computes qkv projection for its heads, the causal
linear-attention bucket scan, and a partial output projection over its 512
attention channels. Host sums the two partials per batch and adds proj_b.

Math notes (vs the reference):
  softmax(q) = exp(q)/sum(exp(q)) without max-subtraction (|q| < ~5, safe).
  attn[tok] = (bq @ ctx) * Dinv with bq = softmax(q)*E^-0.5 and
  Dinv = 1/max(bq @ kc, eps).  The softmax denominator s cancels:
  attn = rawA / rawD with rawA = expq @ ctx, rawD = expq @ kc, except in
  bucket 0 where ctx = kc = 0 and the reference yields exactly 0.
  For buckets >= 1 the eps clamp never binds (rawD is O(1e3)).

Layout: host pre-transposes x to channel-major bf16 blocks and pre-packs
the weights in their SBUF layouts, so every DMA is a straight contiguous
copy.  Per bucket the k^T[v|1] outer product lands in a rotating PSUM slot;
a fp32 running context per head-pair lives in SBUF and is updated by one
DVE add, with a bf16 shadow cast per bucket feeding the next A-matmul.

Numerics: matmuls run in bf16 with fp32 PSUM accumulation; the running
context and the 1/rawD normalization stay in fp32.
"""

import numpy as np

import concourse.bass as bass
import concourse.mybir as mybir
import concourse.tile as tile
from concourse import bacc
from concourse.bass_utils import run_bass_kernel_spmd
from concourse.masks import make_identity

F32 = mybir.dt.float32
BF16 = mybir.dt.bfloat16

B, N, C, H, BS = 4, 4096, 1024, 16, 64
E = C // H                  # 64
HPC = H // 2                # 8 heads per core
QCH = HPC * E               # 512 q (or k or v) channels per core
TB = 512                    # tokens per block
NBLK = N // TB              # 8
NTT = TB // 128             # 4 token-tiles of 128 per block
NBKT = TB // BS             # 8 buckets per block
NIC = C // 128              # 8 input-channel tiles
NQC = QCH // 128            # 4 qkv channel tiles of 128


def emit(nc, reps=1):
    x = nc.dram_tensor("x", (NBLK, 128, NIC, TB), BF16, kind="ExternalInput")
    w = nc.dram_tensor("w", (128, NIC, 3 * QCH), BF16, kind="ExternalInput")
    pw = nc.dram_tensor("pw", (128, NQC, C), BF16, kind="ExternalInput")
    out = nc.dram_tensor("out", (N, C), F32, kind="ExternalOutput")

    with tile.TileContext(nc) as tc:
        with (
            tc.tile_pool(name="persist", bufs=1) as pp,
            tc.tile_pool(name="blk", bufs=3) as bp,
            tc.tile_pool(name="small", bufs=4) as sp,
            tc.tile_pool(name="psBig", bufs=3, space="PSUM") as psA,
            tc.tile_pool(name="psN", bufs=3, space="PSUM") as psC,
            tc.tile_pool(name="psD", bufs=2, space="PSUM") as psD,
        ):
            for _rep in range(reps):
                ident_f32 = pp.tile([128, 128], F32)
                make_identity(nc, ident_f32)
                identb = pp.tile([128, 128], BF16)
                nc.vector.tensor_copy(identb[:], ident_f32[:])

                # weights, contraction dim on partitions, preloaded once
                wt = pp.tile([128, NIC, 3 * QCH], BF16)
                nc.sync.dma_start(wt[:], w[:])
                pwt = pp.tile([128, NQC, C], BF16)
                nc.sync.dma_start(pwt[:], pw[:])

                # running per-head-pair context [ctx | kc], kept in bf16 and
                # advanced by one DVE add per bucket into a fresh pool slot
                # (cb_new = cb_old + outer, outer read from its fp32 PSUM
                # slot), so the serial chain lives on DVE alone and the
                # A-matmuls trail it with a few buckets of slack;
                # head h -> partitions (h%2)*64..+64 of tile h//2
                cbs = [pp.tile([128, E + 1], BF16, name=f"cb{i}") for i in range(4)]
                for c_ in cbs:
                    nc.vector.memset(c_[:], 0.0)
                cbs = list(cbs)  # rebound per bucket to the newest slot

                # [v | 1] tiles: ones column preset once, v cols rewritten per
                # block (manually double-buffered by block parity)
                vs_ab = [
                    pp.tile([128, NTT, HPC, E + 1], BF16, name=f"vs{i}")
                    for i in range(2)
                ]
                for v_ in vs_ab:
                    nc.vector.memset(v_[:], 1.0)

                for blk in range(NBLK):
                    rows = slice(blk * TB, (blk + 1) * TB)
                    xt = bp.tile([128, NIC, TB], BF16)
                    nc.sync.dma_start(xt[:], x[blk])

                    # q^T: (q_ch, tok)
                    eqt = bp.tile([128, NQC, TB], BF16)
                    for qc in range(NQC):
                        qp = psA.tile([128, TB], F32, tag="psA")
                        for ic in range(NIC):
                            nc.tensor.matmul(
                                qp[:],
                                wt[:, ic, qc * 128 : (qc + 1) * 128],
                                xt[:, ic, :],
                                start=(ic == 0),
                                stop=(ic == NIC - 1),
                            )
                        nc.scalar.activation(
                            eqt[:, qc, :], qp[:], mybir.ActivationFunctionType.Exp
                        )

                    # k, v natural: (tok, ch)
                    ek = bp.tile([128, NTT, HPC, E], BF16)
                    vs = vs_ab[blk % 2]
                    for t in range(NTT):
                        kp = psA.tile([128, QCH], F32, tag="psA")
                        vp = psA.tile([128, QCH], F32, tag="psA")
                        for ic in range(NIC):
                            nc.tensor.matmul(
                                kp[:],
                                xt[:, ic, t * 128 : (t + 1) * 128],
                                wt[:, ic, QCH : 2 * QCH],
                                start=(ic == 0),
                                stop=(ic == NIC - 1),
                            )
                        for ic in range(NIC):
                            nc.tensor.matmul(
                                vp[:],
                                xt[:, ic, t * 128 : (t + 1) * 128],
                                wt[:, ic, 2 * QCH : 3 * QCH],
                                start=(ic == 0),
                                stop=(ic == NIC - 1),
                            )
                        nc.scalar.activation(
                            ek[:, t], kp[:], mybir.ActivationFunctionType.Exp
                        )
                        nc.vector.tensor_copy(
                            vs[:, t, :, 0:E],
                            vp[:].rearrange("p (h e) -> p h e", e=E),
                        )

                    # bucket scan: buckets processed in pairs (up)
                    att = bp.tile([128, NQC, TB], BF16)
                    for up in range(NBKT // 2):
                        for hp in range(HPC // 2):
                            qc = hp
                            aps = [
                                psC.tile([128, E + 1], F32, tag="psN", name="ap0"),
                                psC.tile([128, E + 1], F32, tag="psN", name="ap1"),
                            ]
                            cb = cbs[hp]
                            for ui in range(2):
                                ub = 2 * up + ui
                                po = (ub % 2) * 64
                                t = ub // 2
                                first = blk == 0 and ub == 0
                                if not first:
                                    for i, h in enumerate((2 * hp, 2 * hp + 1)):
                                        ho = (h % 2) * 64
                                        nc.tensor.matmul(
                                            aps[i][ui * 64 : ui * 64 + 64, :],
                                            eqt[ho : ho + 64, qc, ub * BS : (ub + 1) * BS],
                                            cb[ho : ho + 64, :],
                                            start=True,
                                            stop=True,
                                        )
                                # outer product expk^T @ [v | 1], both heads
                                dp = psD.tile([128, E + 1], F32, tag="psD", name="dp")
                                for i, h in enumerate((2 * hp, 2 * hp + 1)):
                                    ho = (h % 2) * 64
                                    nc.tensor.matmul(
                                        dp[ho : ho + 64, :],
                                        ek[po : po + 64, t, h, :],
                                        vs[po : po + 64, t, h, :],
                                        start=True,
                                        stop=True,
                                    )
                                cb_new = sp.tile(
                                    [128, E + 1], BF16, tag=f"cb{hp}"
                                )
                                nc.vector.tensor_add(cb_new[:], cb[:], dp[:])
                                cbs[hp] = cb_new
                                cb = cb_new
                            # normalize into column-halves of one tile, then a
                            # single 128x128 transpose + copy for both heads
                            lo = 64 if (blk == 0 and up == 0) else 0
                            anat = sp.tile([128, 128], BF16, tag="anat")
                            if lo:
                                # global bucket 0: attention output is 0
                                nc.vector.memset(anat[0:lo, :], 0.0)
                            for i in range(2):
                                rec = sp.tile([128, 1], F32, tag="rec")
                                nc.vector.reciprocal(
                                    rec[lo:128], aps[i][lo:128, E : E + 1]
                                )
                                nc.scalar.activation(
                                    anat[lo:128, i * E : (i + 1) * E],
                                    aps[i][lo:128, 0:E],
                                    mybir.ActivationFunctionType.Copy,
                                    scale=rec[lo:128],
                                )
                            atp = psC.tile([128, 128], BF16, tag="psN", name="atp")
                            nc.tensor.transpose(atp[:], anat[:], identb[:])
                            nc.vector.tensor_copy(
                                att[:, qc, up * 128 : up * 128 + 128], atp[:]
                            )

                    # output projection (partial over this core's channels)
                    ob = bp.tile([128, NTT, C], F32)
                    for t in range(NTT):
                        for oc in range(2):
                            op = psA.tile([128, 512], F32, tag="psA")
                            for qc in range(NQC):
                                nc.tensor.matmul(
                                    op[:],
                                    att[:, qc, t * 128 : (t + 1) * 128],
                                    pwt[:, qc, oc * 512 : (oc + 1) * 512],
                                    start=(qc == 0),
                                    stop=(qc == NQC - 1),
                                )
                            nc.vector.tensor_copy(
                                ob[:, t, oc * 512 : (oc + 1) * 512], op[:]
                            )
                    nc.sync.dma_start(
                        out[rows, :].rearrange("(t p) c -> p t c", p=128),
                        ob[:],
                    )
    return nc


_NC_CACHE = {}


def _get_nc():
    if "nc" not in _NC_CACHE:
        nc = bacc.Bacc("TRN2", target_bir_lowering=False, debug=False, num_devices=8)
        emit(nc)
        nc.compile()
        _NC_CACHE["nc"] = nc
    return _NC_CACHE["nc"]


def kernel(x, qkv_w, proj_w, proj_b, _trace=False):
    import ml_dtypes

    bf16 = ml_dtypes.bfloat16
    x = np.asarray(x, dtype=np.float32)
    qkv_w = np.asarray(qkv_w, dtype=np.float32)
    proj_w = np.asarray(proj_w, dtype=np.float32)
    proj_b = np.asarray(proj_b, dtype=np.float32)

    wT = qkv_w.T.astype(bf16)               # (C, 3C)
    pT = proj_w.T.astype(bf16)              # (C, C)

    in_maps = []
    for core in range(8):
        b, hg = core // 2, core % 2
        s = hg * QCH
        # (C, 3*QCH) -> SBUF layout (128, NIC, 3*QCH)
        wcore = np.concatenate(
            [wT[:, s : s + QCH], wT[:, C + s : C + s + QCH], wT[:, 2 * C + s : 2 * C + s + QCH]],
            axis=1,
        )
        wl = np.ascontiguousarray(
            wcore.reshape(NIC, 128, 3 * QCH).transpose(1, 0, 2)
        )
        # (QCH, C) -> SBUF layout (128, NQC, C)
        pl = np.ascontiguousarray(
            pT[s : s + QCH, :].reshape(NQC, 128, C).transpose(1, 0, 2)
        )
        # x[b] (N, C) -> per-block channel-major (NBLK, 128, NIC, TB)
        xb = x[b].astype(bf16)              # (N, C)
        xl = np.ascontiguousarray(
            xb.reshape(NBLK, TB, NIC, 128).transpose(0, 3, 2, 1)
        )
        in_maps.append({"x": xl, "w": wl, "pw": pl})

    nc = _get_nc()
    res = run_bass_kernel_spmd(nc, in_maps, core_ids=list(range(8)), trace=_trace)
    outs = [res.results[c]["out"] for c in range(8)]
    full = np.empty((B, N, C), dtype=np.float32)
    for b in range(B):
        full[b] = outs[2 * b] + outs[2 * b + 1] + proj_b[None, :]
    if _trace:
        return full, res
    return full


# revision 20
# speedup vs baseline: 1.0595x; 1.0595x over previous
"""Bucketed causal-linear attention kernel for Trainium2 (8 NeuronCores).

Sharding: core c handles batch b = c//2 and head-group hg = c%2 (8 of 16
heads). Each core computes qkv projection for its heads, the causal
linear-attention bucket scan, and a partial output projection over its 512
attention channels. Host sums the two partials per batch and adds proj_b.

Math notes (vs the reference):
  softmax(q) = exp(q)/sum(exp(q)) without max-subtraction (|q| < ~5, safe).
  attn[tok] = (bq @ ctx) * Dinv with bq = softmax(q)*E^-0.5 and
  Dinv = 1/max(bq @ kc, eps).  The softmax denominator s cancels:
  attn = rawA / rawD with rawA = expq @ ctx, rawD = expq @ kc, except in
  bucket 0 where ctx = kc = 0 and the reference yields exactly 0.
  For buckets >= 1 the eps clamp never binds (rawD is O(1e3)).

Layout: host pre-transposes x to channel-major bf16 blocks and pre-packs
the weights in their SBUF layouts, so every DMA is a straight contiguous
copy.  Per bucket the k^T[v|1] outer product lands in a rotating PSUM slot;
a fp32 running context per head-pair lives in SBUF and is updated by one
DVE add, with a bf16 shadow cast per bucket feeding the next A-matmul.

Numerics: matmuls run in bf16 with fp32 PSUM accumulation; the running
context and the 1/rawD normalization stay in fp32.
"""

import numpy as np

import concourse.bass as bass
import concourse.mybir as mybir
import concourse.tile as tile
from concourse import bacc
from concourse.bass_utils import run_bass_kernel_spmd
from concourse.masks import make_identity

F32 = mybir.dt.float32
BF16 = mybir.dt.bfloat16

B, N, C, H, BS = 4, 4096, 1024, 16, 64
E = C // H                  # 64
HPC = H // 2                # 8 heads per core
QCH = HPC * E               # 512 q (or k or v) channels per core
TB = 512                    # tokens per block
NBLK = N // TB              # 8
NTT = TB // 128             # 4 token-tiles of 128 per block
NBKT = TB // BS             # 8 buckets per block
NIC = C // 128              # 8 input-channel tiles
NQC = QCH // 128            # 4 qkv channel tiles of 128


def emit(nc, reps=1):
    x = nc.dram_tensor("x", (NBLK, 128, NIC, TB), BF16, kind="ExternalInput")
    w = nc.dram_tensor("w", (128, NIC, 3 * QCH), BF16, kind="ExternalInput")
    pw = nc.dram_tensor("pw", (128, NQC, C), BF16, kind="ExternalInput")
    out = nc.dram_tensor("out", (N, C), F32, kind="ExternalOutput")

    with tile.TileContext(nc) as tc:
        with (
            tc.tile_pool(name="persist", bufs=1) as pp,
            tc.tile_pool(name="blk", bufs=3) as bp,
            tc.tile_pool(name="small", bufs=4) as sp,
            tc.tile_pool(name="psBig", bufs=3, space="PSUM") as psA,
            tc.tile_pool(name="psN", bufs=3, space="PSUM") as psC,
            tc.tile_pool(name="psD", bufs=2, space="PSUM") as psD,
        ):
            for _rep in range(reps):
                ident_f32 = pp.tile([128, 128], F32)
                make_identity(nc, ident_f32)
                identb = pp.tile([128, 128], BF16)
                nc.vector.tensor_copy(identb[:], ident_f32[:])

                # weights, contraction dim on partitions, preloaded once
                wt = pp.tile([128, NIC, 3 * QCH], BF16)
                nc.sync.dma_start(wt[:], w[:])
                pwt = pp.tile([128, NQC, C], BF16)
                nc.sync.dma_start(pwt[:], pw[:])

                # running per-head-pair context [ctx | kc], kept in bf16 and
                # advanced by one DVE add per bucket into a fresh pool slot
                # (cb_new = cb_old + outer, outer read from its fp32 PSUM
                # slot), so the serial chain lives on DVE alone and the
                # A-matmuls trail it with a few buckets of slack;
                # head h -> partitions (h%2)*64..+64 of tile h//2
                cbs = [pp.tile([128, E + 1], BF16, name=f"cb{i}") for i in range(4)]
                for c_ in cbs:
                    nc.vector.memset(c_[:], 0.0)
                cbs = list(cbs)  # rebound per bucket to the newest slot

                # [v | 1] tiles: ones column preset once, v cols rewritten per
                # block (manually double-buffered by block parity)
                vs_ab = [
                    pp.tile([128, NTT, HPC, E + 1], BF16, name=f"vs{i}")
                    for i in range(2)
                ]
                for v_ in vs_ab:
                    nc.vector.memset(v_[:], 1.0)

                for blk in range(NBLK):
                    rows = slice(blk * TB, (blk + 1) * TB)
                    xt = bp.tile([128, NIC, TB], BF16)
                    nc.sync.dma_start(xt[:], x[blk])

                    # q^T: (q_ch, tok)
                    eqt = bp.tile([128, NQC, TB], BF16)
                    for qc in range(NQC):
                        qp = psA.tile([128, TB], F32, tag="psA")
                        for ic in range(NIC):
                            nc.tensor.matmul(
                                qp[:],
                                wt[:, ic, qc * 128 : (qc + 1) * 128],
                                xt[:, ic, :],
                                start=(ic == 0),
                                stop=(ic == NIC - 1),
                            )
                        nc.scalar.activation(
                            eqt[:, qc, :], qp[:], mybir.ActivationFunctionType.Exp
                        )

                    # k, v natural: (tok, ch)
                    ek = bp.tile([128, NTT, HPC, E], BF16)
                    vs = vs_ab[blk % 2]
                    for t in range(NTT):
                        kp = psA.tile([128, QCH], F32, tag="psA")
                        vp = psA.tile([128, QCH], F32, tag="psA")
                        for ic in range(NIC):
                            nc.tensor.matmul(
                                kp[:],
                                xt[:, ic, t * 128 : (t + 1) * 128],
                                wt[:, ic, QCH : 2 * QCH],
                                start=(ic == 0),
                                stop=(ic == NIC - 1),
                            )
                        for ic in range(NIC):
                            nc.tensor.matmul(
                                vp[:],
                                xt[:, ic, t * 128 : (t + 1) * 128],
                                wt[:, ic, 2 * QCH : 3 * QCH],
                                start=(ic == 0),
                                stop=(ic == NIC - 1),
                            )
                        nc.scalar.activation(
                            ek[:, t], kp[:], mybir.ActivationFunctionType.Exp
                        )
                        nc.vector.tensor_copy(
                            vs[:, t, :, 0:E],
                            vp[:].rearrange("p (h e) -> p h e", e=E),
                        )

                    # bucket scan: buckets processed in pairs (up)
                    att = bp.tile([128, NQC, TB], BF16)
                    for up in range(NBKT // 2):
                        for hp in range(HPC // 2):
                            qc = hp
                            aps = [
                                psC.tile([128, E + 1], F32, tag="psN", name="ap0"),
                                psC.tile([128, E + 1], F32, tag="psN", name="ap1"),
                            ]
                            cb = cbs[hp]
                            for ui in range(2):
                                ub = 2 * up + ui
                                po = (ub % 2) * 64
                                t = ub // 2
                                first = blk == 0 and ub == 0
                                if not first:
                                    for i, h in enumerate((2 * hp, 2 * hp + 1)):
                                        ho = (h % 2) * 64
                                        nc.tensor.matmul(
                                            aps[i][ui * 64 : ui * 64 + 64, :],
                                            eqt[ho : ho + 64, qc, ub * BS : (ub + 1) * BS],
                                            cb[ho : ho + 64, :],
                                            start=True,
                                            stop=True,
                                        )
                                # outer product expk^T @ [v | 1], both heads
                                dp = psD.tile([128, E + 1], F32, tag="psD", name="dp")
                                for i, h in enumerate((2 * hp, 2 * hp + 1)):
                                    ho = (h % 2) * 64
                                    nc.tensor.matmul(
                                        dp[ho : ho + 64, :],
                                        ek[po : po + 64, t, h, :],
                                        vs[po : po + 64, t, h, :],
                                        start=True,
                                        stop=True,
                                    )
                                cb_new = sp.tile(
                                    [128, E + 1], BF16, tag=f"cb{hp}"
                                )
                                nc.vector.tensor_add(cb_new[:], cb[:], dp[:])
                                cbs[hp] = cb_new
                                cb = cb_new
                            # normalize into column-halves of one tile, then a
                            # single 128x128 transpose + copy for both heads
                            lo = 64 if (blk == 0 and up == 0) else 0
                            anat = sp.tile([128, 128], BF16, tag="anat")
                            if lo:
                                # global bucket 0: attention output is 0
                                nc.vector.memset(anat[0:lo, :], 0.0)
                            for i in range(2):
                                rec = sp.tile([128, 1], F32, tag="rec")
                                nc.vector.reciprocal(
                                    rec[lo:128], aps[i][lo:128, E : E + 1]
                                )
                                nc.vector.tensor_scalar_mul(
                                    anat[lo:128, i * E : (i + 1) * E],
                                    aps[i][lo:128, 0:E],
                                    rec[lo:128],
                                )
                            atp = psC.tile([128, 128], BF16, tag="psN", name="atp")
                            nc.tensor.transpose(atp[:], anat[:], identb[:])
                            nc.vector.tensor_copy(
                                att[:, qc, up * 128 : up * 128 + 128], atp[:]
                            )

                    # output projection (partial over this core's channels)
                    ob = bp.tile([128, NTT, C], F32)
                    for t in range(NTT):
                        for oc in range(2):
                            op = psA.tile([128, 512], F32, tag="psA")
                            for qc in range(NQC):
                                nc.tensor.matmul(
                                    op[:],
                                    att[:, qc, t * 128 : (t + 1) * 128],
                                    pwt[:, qc, oc * 512 : (oc + 1) * 512],
                                    start=(qc == 0),
                                    stop=(qc == NQC - 1),
                                )
                            nc.vector.tensor_copy(
                                ob[:, t, oc * 512 : (oc + 1) * 512], op[:]
                            )
                    nc.sync.dma_start(
                        out[rows, :].rearrange("(t p) c -> p t c", p=128),
                        ob[:],
                    )
    return nc


_NC_CACHE = {}


def _get_nc():
    if "nc" not in _NC_CACHE:
        nc = bacc.Bacc("TRN2", target_bir_lowering=False, debug=False, num_devices=8)
        emit(nc)
        nc.compile()
        _NC_CACHE["nc"] = nc
    return _NC_CACHE["nc"]


def kernel(x, qkv_w, proj_w, proj_b, _trace=False):
    import ml_dtypes

    bf16 = ml_dtypes.bfloat16
    x = np.asarray(x, dtype=np.float32)
    qkv_w = np.asarray(qkv_w, dtype=np.float32)
    proj_w = np.asarray(proj_w, dtype=np.float32)
    proj_b = np.asarray(proj_b, dtype=np.float32)

    wT = qkv_w.T.astype(bf16)               # (C, 3C)
    pT = proj_w.T.astype(bf16)              # (C, C)

    in_maps = []
    for core in range(8):
        b, hg = core // 2, core % 2
        s = hg * QCH
        # (C, 3*QCH) -> SBUF layout (128, NIC, 3*QCH)
        wcore = np.concatenate(
            [wT[:, s : s + QCH], wT[:, C + s : C + s + QCH], wT[:, 2 * C + s : 2 * C + s + QCH]],
            axis=1,
        )
        wl = np.ascontiguousarray(
            wcore.reshape(NIC, 128, 3 * QCH).transpose(1, 0, 2)
        )
        # (QCH, C) -> SBUF layout (128, NQC, C)
        pl = np.ascontiguousarray(
            pT[s : s + QCH, :].reshape(NQC, 128, C).transpose(1, 0, 2)
        )
        # x[b] (N, C) -> per-block channel-major (NBLK, 128, NIC, TB)
        xb = x[b].astype(bf16)              # (N, C)
        xl = np.ascontiguousarray(
            xb.reshape(NBLK, TB, NIC, 128).transpose(0, 3, 2, 1)
        )
        in_maps.append({"x": xl, "w": wl, "pw": pl})

    nc = _get_nc()
    res = run_bass_kernel_spmd(nc, in_maps, core_ids=list(range(8)), trace=_trace)
    outs = [res.results[c]["out"] for c in range(8)]
    full = np.empty((B, N, C), dtype=np.float32)
    for b in range(B):
        full[b] = outs[2 * b] + outs[2 * b + 1] + proj_b[None, :]
    if _trace:
        return full, res
    return full
